# revision 21
# baseline (speedup 1.0000x reference)
"""Trainium2 Bass kernel: single-head causal attention (B=8, T=2048, E=1024, H=64).

Sharding: data-parallel over the batch dim — one batch element per NeuronCore,
8 cores, no collectives.

Per-core pipeline (matmuls in float32r — full PE rate at N>=256):
  phase A: DMA X [T,E] by 128-row tiles; PE-transpose (fp32) into XT [E,T]
           chunks, rounded to f32r during the PSUM->SBUF copy;
           projections QK^T = [Wq|Wk]^T @ X^T (one M=128 matmul) and
           V^T = Wv^T @ X^T, accumulated over 8 e-chunks per 512-col t-chunk.
           Biases fused into the PSUM->SBUF copies. V^T is PE-transposed back
           to natural [t, H] layout with a ones column appended (V').
  phase B: per 128-wide k-chunk j: S^T[k, q] = K_j Q^T for q >= 128j only
           (causal skip); the causal -1e30 mask is added to the diagonal
           128x128 block in PSUM by DVE; exp applied by ScalarE directly from
           PSUM with the 1/sqrt(H) scale fused; P~V accumulated into O' PSUM
           [65, q] where row 64 (the ones column of V') accumulates the
           softmax denominator Z.
  Output per core: [65, 2048] = [unnormalized O^T; Z]. Host divides by Z and
  transposes during the unshard (part of gather).

Hardware quirks catered to:
  - f32r matmul operands must be produced rounded-to-f32r (verifier rule), so
    matmul-feeding tiles are f32r and the producing copy/activation rounds.
  - PE instructions can carry only ONE semaphore wait (walrus codegen limit),
    so all constants ship as one host-packed block via a single DMA, K^T is
    re-based with one DMA, and dummy PE ops make the PE "observe" DMA/DVE
    ticks at phase boundaries so real matmuls need at most one wait each.
"""

import numpy as np

import concourse.bass as bass
import concourse.bacc as bacc
import concourse.mybir as mybir
from concourse.tile import TileContext
from concourse.bass_utils import run_bass_kernel_spmd

T = 2048
E = 1024
H = 64
P = 128
TC = 512  # t/q chunk width (one PSUM bank of f32)
NT = T // P  # 16 t-tiles
NE = E // P  # 8 e-chunks
NTC = T // TC  # 4 t-chunks
NCORES = 8

F32 = mybir.dt.float32
F32R = mybir.dt.float32r
AF = mybir.ActivationFunctionType

# constant block column layout (per partition)
CB_IDENT = 0  # [128] identity
CB_WQK = CB_IDENT + P  # [NE * 2H] = 1024, [e_chunk, m] with m: 0:64=Wq, 64:128=Wk
CB_WV = CB_WQK + NE * 2 * H  # [NE * H] = 512
CB_MASK = CB_WV + NE * H  # [128] causal mask: 0 keep (y>=p), -1e30 drop
CB_BQK = CB_MASK + P  # [1] bq on partitions 0:64, bk on 64:128
CB_BV = CB_BQK + 1  # [1] bv on partitions 0:64
CB_COLS = CB_BV + 1


def pack_const_block(Wq, Wk, Wv, bq, bk, bv):
    cb = np.zeros((P, CB_COLS), dtype=np.float32)
    cb[:, CB_IDENT : CB_IDENT + P] = np.eye(P, dtype=np.float32)
    wqk = np.zeros((P, NE, 2 * H), dtype=np.float32)
    wqk[:, :, 0:H] = Wq.reshape(NE, P, H).transpose(1, 0, 2)
    wqk[:, :, H : 2 * H] = Wk.reshape(NE, P, H).transpose(1, 0, 2)
    cb[:, CB_WQK : CB_WQK + NE * 2 * H] = wqk.reshape(P, NE * 2 * H)
    cb[:, CB_WV : CB_WV + NE * H] = (
        Wv.reshape(NE, P, H).transpose(1, 0, 2).reshape(P, NE * H)
    )
    p_idx = np.arange(P)[:, None]
    y_idx = np.arange(P)[None, :]
    cb[:, CB_MASK : CB_MASK + P] = np.where(y_idx >= p_idx, 0.0, -1e30).astype(
        np.float32
    )
    cb[0:H, CB_BQK] = bq
    cb[H : 2 * H, CB_BQK] = bk
    cb[0:H, CB_BV] = bv
    return cb


def build_kernel():
    nc = bacc.Bacc("TRN2", target_bir_lowering=False, debug=False)
    x = nc.dram_tensor("x", [T, E], F32, kind="ExternalInput")
    cb = nc.dram_tensor("cb", [P, CB_COLS], F32R, kind="ExternalInput")
    out = nc.dram_tensor("out", [H + 1, T], F32, kind="ExternalOutput")

    with TileContext(nc) as tc:
        with tc.tile_pool(name="const", bufs=1) as const:
            cb_sb = const.tile([P, CB_COLS], F32R)
            nc.gpsimd.dma_start(cb_sb[:], cb[:])
            ident = cb_sb[:, CB_IDENT : CB_IDENT + P].bitcast(F32)
            wqk_sb = cb_sb[:, CB_WQK : CB_WQK + NE * 2 * H].rearrange(
                "p (c m) -> p c m", m=2 * H
            )
            wv_sb = cb_sb[:, CB_WV : CB_WV + NE * H].rearrange(
                "p (c m) -> p c m", m=H
            )
            maskneg = cb_sb[:, CB_MASK : CB_MASK + P].bitcast(F32)
            bqk_t = cb_sb[:, CB_BQK : CB_BQK + 1].bitcast(F32)
            bv_t = cb_sb[0:H, CB_BV : CB_BV + 1].bitcast(F32)

            # persistent activations
            qk_sb = const.tile([P, T], F32R)  # rows 0:64 = Q^T, 64:128 = K^T
            kt_sb = const.tile([H, T], F32R)  # K^T re-based to partitions 0:64
            vt_sb = const.tile([H, T], F32)  # V^T staging
            v_sb = const.tile([P, NT, H + 1], F32R)  # V' = [V, 1] natural layout
            o_sb = const.tile([H + 1, T], F32)
            # ones column of V' (memset can't write f32r; 0*x+1 rounds via DVE)
            nc.vector.tensor_scalar(
                v_sb[:, :, H],
                cb_sb[:, 0:NT],
                0.0,
                1.0,
                mybir.AluOpType.mult,
                mybir.AluOpType.add,
            )

            # ---------------- phase A: transpose X + projections ----------------
            with (
                tc.tile_pool(name="xin", bufs=6) as xpool,
                tc.tile_pool(name="xt", bufs=2) as xtpool,
                tc.tile_pool(name="ps_xt", bufs=3, space="PSUM") as ps_xt,
                tc.tile_pool(name="ps_prj", bufs=2, space="PSUM") as ps_prj,
                tc.tile_pool(name="ps_v", bufs=1, space="PSUM") as ps_v,
            ):
                # PE observes the const-block DMA once, so later PE ops don't
                # need a second wait slot for it
                warm = ps_xt.tile([P, 4 * P], F32, tag="pxt")
                nc.tensor.matmul(
                    warm[:, 0:P],
                    cb_sb[:, 0:P],
                    cb_sb[:, 0:P],
                    start=True,
                    stop=True,
                )
                for tcn in range(NTC):
                    xts = []
                    for tt in range(4):
                        xt_in = xpool.tile([P, E], F32, tag="x")
                        t0 = (tcn * 4 + tt) * P
                        nc.sync.dma_start(xt_in[:], x[t0 : t0 + P, :])
                        xts.append(xt_in)
                    xt_sb = xtpool.tile([P, NE, TC], F32R, tag="xt")
                    for ec in range(NE):
                        pxt = ps_xt.tile([P, 4 * P], F32, tag="pxt")
                        for tt in range(4):
                            nc.tensor.transpose(
                                pxt[:, tt * P : (tt + 1) * P],
                                xts[tt][:, ec * P : (ec + 1) * P],
                                ident,
                            )
                        nc.vector.tensor_copy(xt_sb[:, ec, :], pxt[:])
                    pqk = ps_prj.tile([P, TC], F32, tag="pqk")
                    pv = ps_prj.tile([H, TC], F32, tag="pv")
                    for ec in range(NE):
                        nc.tensor.matmul(
                            pqk[:],
                            wqk_sb[:, ec, :],
                            xt_sb[:, ec, :],
                            start=(ec == 0),
                            stop=(ec == NE - 1),
                        )
                    for ec in range(NE):
                        nc.tensor.matmul(
                            pv[:],
                            wv_sb[:, ec, :],
                            xt_sb[:, ec, :],
                            start=(ec == 0),
                            stop=(ec == NE - 1),
                        )
                    c0 = tcn * TC
                    nc.vector.tensor_scalar_add(qk_sb[:, c0 : c0 + TC], pqk[:], bqk_t)
                    nc.vector.tensor_scalar_add(vt_sb[:, c0 : c0 + TC], pv[:], bv_t)
                    for tt in range(4):
                        ti = tcn * 4 + tt
                        psv = ps_v.tile([P, H], F32, tag="psv")
                        nc.tensor.transpose(
                            psv[:],
                            vt_sb[:, ti * P : (ti + 1) * P],
                            ident[0:H, 0:H],
                        )
                        nc.vector.tensor_copy(v_sb[:, ti, 0:H], psv[:])
                # one DMA for the K^T re-base: all phase-B S matmuls then
                # depend on a single queue semaphore
                nc.gpsimd.dma_start(kt_sb[:], qk_sb[H : 2 * H, :])

            # ---------------- phase B: scores, softmax, PV ----------------
            with (
                tc.tile_pool(name="es", bufs=3) as espool,
                tc.tile_pool(name="ps_s", bufs=3, space="PSUM") as ps_s,
                tc.tile_pool(name="ps_o", bufs=1, space="PSUM") as ps_o,
            ):
                # PE observes the last phase-A DVE tick (psv copy of tile 15),
                # covering the PSUM WAR deps of the re-used banks below
                bridge = ps_s.tile([P, TC], F32, tag="s")
                nc.tensor.matmul(
                    bridge[0:H, 0:H],
                    v_sb[:, NT - 1, 0:H],
                    v_sb[:, NT - 1, 0:H],
                    start=True,
                    stop=True,
                )
                o_ps = [
                    ps_o.tile([H + 1, TC], F32, tag=f"o{c}", name=f"o_ps{c}")
                    for c in range(NTC)
                ]
                scale = 1.0 / np.sqrt(np.float32(H))
                for j in range(NT):
                    k0 = j * P
                    es = espool.tile([P, T], F32R, tag="es")
                    for c in range(j // 4, NTC):
                        q0 = max(c * TC, k0)
                        w = (c + 1) * TC - q0
                        ps = ps_s.tile([P, TC], F32, tag="s")
                        nc.tensor.matmul(
                            ps[:, :w],
                            kt_sb[:, k0 : k0 + P],
                            qk_sb[0:H, q0 : q0 + w],
                            start=True,
                            stop=True,
                        )
                        if c == j // 4:
                            # causal mask inside the diagonal 128x128 block:
                            # add -1e30 where y < p, in PSUM, before the exp
                            nc.vector.tensor_tensor(
                                ps[:, 0:P],
                                ps[:, 0:P],
                                maskneg,
                                mybir.AluOpType.add,
                            )
                        nc.scalar.activation(
                            es[:, q0 : q0 + w], ps[:, :w], AF.Exp, scale=float(scale)
                        )
                    for c in range(j // 4, NTC):
                        q0 = max(c * TC, k0)
                        w = (c + 1) * TC - q0
                        nc.tensor.matmul(
                            o_ps[c][:, q0 - c * TC : q0 - c * TC + w],
                            v_sb[:, j, :],
                            es[:, q0 : q0 + w],
                            start=(j == 0),
                            stop=(j == 4 * c + 3),
                        )
                        if j == 4 * c + 3:
                            nc.vector.tensor_copy(
                                o_sb[:, c * TC : (c + 1) * TC], o_ps[c][:]
                            )
                            nc.sync.dma_start(
                                out[:, c * TC : (c + 1) * TC],
                                o_sb[:, c * TC : (c + 1) * TC],
                            )
    nc.compile()
    return nc


_NC_CACHE = None


def _get_nc():
    global _NC_CACHE
    if _NC_CACHE is None:
        _NC_CACHE = build_kernel()
    return _NC_CACHE


def kernel(batch_x, Wk, bk, Wq, bq, Wv, bv):
    batch_x = np.ascontiguousarray(np.asarray(batch_x, dtype=np.float32))
    cb = pack_const_block(
        np.asarray(Wq, dtype=np.float32),
        np.asarray(Wk, dtype=np.float32),
        np.asarray(Wv, dtype=np.float32),
        np.asarray(bq, dtype=np.float32),
        np.asarray(bk, dtype=np.float32),
        np.asarray(bv, dtype=np.float32),
    )
    nc = _get_nc()
    in_maps = [{"x": batch_x[i], "cb": cb} for i in range(NCORES)]
    res = run_bass_kernel_spmd(nc, in_maps, list(range(NCORES)))
    outs = []
    for i in range(NCORES):
        o = res.results[i]["out"]  # [65, 2048]
        outs.append((o[:H] / o[H : H + 1]).T)  # normalize + transpose
    return np.stack(outs).astype(np.float32)


if __name__ == "__main__":
    rng = np.random.default_rng(0)
    inputs = {
        "batch_x": rng.standard_normal((NCORES, T, E), dtype=np.float32),
        "Wk": rng.standard_normal((E, H), dtype=np.float32) * 0.03,
        "bk": rng.standard_normal((H,), dtype=np.float32) * 0.03,
        "Wq": rng.standard_normal((E, H), dtype=np.float32) * 0.03,
        "bq": rng.standard_normal((H,), dtype=np.float32) * 0.03,
        "Wv": rng.standard_normal((E, H), dtype=np.float32) * 0.03,
        "bv": rng.standard_normal((H,), dtype=np.float32) * 0.03,
    }
    out = kernel(**inputs)
    print(out.shape, out.dtype)


# revision 23
# speedup vs baseline: 1.0925x; 1.0925x over previous
"""Trainium2 Bass kernel: single-head causal attention (B=8, T=2048, E=1024, H=64).

Sharding: data-parallel over the batch dim — one batch element per NeuronCore,
8 cores, no collectives.

Per-core pipeline (matmuls in float32r — full PE rate at N>=256):
  phase A: DMA X [T,E] by 128-row tiles; PE-transpose (f32r, 1.5 cyc/row) into
           XT [E,T] chunks (PSUM->SBUF copies split DVE/ScalarE);
           projections QK^T = [Wq|Wk]^T @ X^T (one M=128 matmul) and
           V^T = Wv^T @ X^T run one t-chunk BEHIND the transposes so the PE
           never waits on the copies. Biases fused into the PSUM->SBUF copies.
           V^T is PE-transposed back to natural [t, H] layout with a ones
           column appended (V').
  phase B: software-pipelined per 128-wide k-chunk j: S^T_j = K_j Q^T for
           q >= 128j (causal skip) while PV of chunk j-1 runs, so the PE never
           waits on the exp. Causal -1e30 mask added to the diagonal 128x128
           block in PSUM by DVE pre-exp; exp on ScalarE straight from PSUM
           with the 1/sqrt(H) scale fused; P~V accumulates into O' PSUM
           [65, q] whose row 64 (ones column of V') is the denominator Z.
  Output per core: [65, 2048] = [unnormalized O^T; Z]. Host divides by Z and
  transposes during the unshard (part of gather).

Hardware quirks catered to:
  - f32r matmul operands must be produced rounded-to-f32r (verifier rule);
    DMA f32r->f32r is accepted, so x/cb ship as f32r from the host.
  - f32r ISA restrictions: moving-operand and dst innermost counts even,
    dst 8B-aligned at partition 0.
  - PE matmuls carry only ONE semaphore wait (walrus limit) — bacc's
    generate_event_semaphores splits the rest, but the kernel is structured
    to keep cross-engine fan-in low anyway.
  - A warmup matmul burst during the DMA prologue brings the PE HAM clock
    gate to 2.4 GHz before the real work starts.
"""

import numpy as np

import concourse.bass as bass
import concourse.bacc as bacc
import concourse.mybir as mybir
from concourse.tile import TileContext
from concourse.bass_utils import run_bass_kernel_spmd

T = 2048
E = 1024
H = 64
P = 128
TC = 512  # t/q chunk width (one PSUM bank of f32)
NT = T // P  # 16 t-tiles
NE = E // P  # 8 e-chunks
NTC = T // TC  # 4 t-chunks
NCORES = 8

F32 = mybir.dt.float32
F32R = mybir.dt.float32r
AF = mybir.ActivationFunctionType

# constant block column layout (per partition)
CB_IDENT = 0  # [128] identity
CB_WQK = CB_IDENT + P  # [NE * 2H] = 1024, [e_chunk, m] with m: 0:64=Wq, 64:128=Wk
CB_WV = CB_WQK + NE * 2 * H  # [NE * H] = 512
CB_MASK = CB_WV + NE * H  # [128] causal mask: 0 keep (y>=p), -1e30 drop
CB_BQK = CB_MASK + P  # [1] bq on partitions 0:64, bk on 64:128
CB_BV = CB_BQK + 1  # [1] bv on partitions 0:64
CB_COLS = CB_BV + 1


def pack_const_block(Wq, Wk, Wv, bq, bk, bv):
    cb = np.zeros((P, CB_COLS), dtype=np.float32)
    cb[:, CB_IDENT : CB_IDENT + P] = np.eye(P, dtype=np.float32)
    wqk = np.zeros((P, NE, 2 * H), dtype=np.float32)
    wqk[:, :, 0:H] = Wq.reshape(NE, P, H).transpose(1, 0, 2)
    wqk[:, :, H : 2 * H] = Wk.reshape(NE, P, H).transpose(1, 0, 2)
    cb[:, CB_WQK : CB_WQK + NE * 2 * H] = wqk.reshape(P, NE * 2 * H)
    cb[:, CB_WV : CB_WV + NE * H] = (
        Wv.reshape(NE, P, H).transpose(1, 0, 2).reshape(P, NE * H)
    )
    p_idx = np.arange(P)[:, None]
    y_idx = np.arange(P)[None, :]
    cb[:, CB_MASK : CB_MASK + P] = np.where(y_idx >= p_idx, 0.0, -1e30).astype(
        np.float32
    )
    cb[0:H, CB_BQK] = bq
    cb[H : 2 * H, CB_BQK] = bk
    cb[0:H, CB_BV] = bv
    return cb


def build_kernel():
    nc = bacc.Bacc("TRN2", target_bir_lowering=False, debug=False)
    x = nc.dram_tensor("x", [T, E], F32R, kind="ExternalInput")
    cb = nc.dram_tensor("cb", [P, CB_COLS], F32R, kind="ExternalInput")
    out = nc.dram_tensor("out", [H + 1, T], F32, kind="ExternalOutput")

    with TileContext(nc) as tc:
        with tc.tile_pool(name="const", bufs=1) as const:
            cb_sb = const.tile([P, CB_COLS], F32R)
            nc.gpsimd.dma_start(cb_sb[:], cb[:])
            ident = cb_sb[:, CB_IDENT : CB_IDENT + P]  # f32r identity
            wqk_sb = cb_sb[:, CB_WQK : CB_WQK + NE * 2 * H].rearrange(
                "p (c m) -> p c m", m=2 * H
            )
            wv_sb = cb_sb[:, CB_WV : CB_WV + NE * H].rearrange(
                "p (c m) -> p c m", m=H
            )
            maskneg = cb_sb[:, CB_MASK : CB_MASK + P].bitcast(F32)
            bqk_t = cb_sb[:, CB_BQK : CB_BQK + 1].bitcast(F32)
            bv_t = cb_sb[0:H, CB_BV : CB_BV + 1].bitcast(F32)

            # persistent activations
            qk_sb = const.tile([P, T], F32R)  # rows 0:64 = Q^T, 64:128 = K^T
            kt_sb = const.tile([H, T], F32R)  # K^T re-based to partitions 0:64
            vt_sb = const.tile([H, T], F32R)  # V^T staging
            v_sb = const.tile([P, NT, H + 1], F32R)  # V' = [V, 1] natural layout
            o_sb = const.tile([H + 1, T], F32)
            # ones column of V' (memset can't write f32r; 0*x+1 rounds via DVE)
            nc.vector.tensor_scalar(
                v_sb[:, :, H],
                cb_sb[:, 0:NT],
                0.0,
                1.0,
                mybir.AluOpType.mult,
                mybir.AluOpType.add,
            )

            # ---------------- phase A: transpose X + projections ----------------
            with (
                tc.tile_pool(name="xin", bufs=9) as xpool,
                tc.tile_pool(name="xt", bufs=2) as xtpool,
                tc.tile_pool(name="ps_xt", bufs=3, space="PSUM") as ps_xt,
                tc.tile_pool(name="ps_prj", bufs=2, space="PSUM") as ps_prj,
                tc.tile_pool(name="ps_v", bufs=1, space="PSUM") as ps_v,
            ):
                # HAM warmup: ~4.5us of back-to-back matmuls while the first
                # x tiles stream in, so real work starts at 2.4 GHz
                warm = ps_xt.tile([P, 4 * P], F32, tag="pxt")
                for _ in range(42):
                    nc.tensor.matmul(
                        warm[:, 0:P],
                        cb_sb[:, 0:P],
                        cb_sb[:, 0:P],
                        start=True,
                        stop=True,
                    )

                def emit_proj(tcn, xt_sb):
                    pqk = ps_prj.tile([P, TC], F32, tag="pqk", name=f"pqk{tcn}")
                    pv = ps_prj.tile([H, TC], F32, tag="pv", name=f"pv{tcn}")
                    for ec in range(NE):
                        nc.tensor.matmul(
                            pqk[:],
                            wqk_sb[:, ec, :],
                            xt_sb[:, ec, :],
                            start=(ec == 0),
                            stop=(ec == NE - 1),
                        )
                    for ec in range(NE):
                        nc.tensor.matmul(
                            pv[:],
                            wv_sb[:, ec, :],
                            xt_sb[:, ec, :],
                            start=(ec == 0),
                            stop=(ec == NE - 1),
                        )
                    c0 = tcn * TC
                    nc.vector.tensor_scalar_add(qk_sb[:, c0 : c0 + TC], pqk[:], bqk_t)
                    nc.vector.tensor_scalar_add(vt_sb[:, c0 : c0 + TC], pv[:], bv_t)
                    for tt in range(4):
                        ti = tcn * 4 + tt
                        psv = ps_v.tile([P, H], F32, tag="psv", name=f"psv{ti}")
                        nc.tensor.transpose(
                            psv[:].bitcast(F32R),
                            vt_sb[:, ti * P : (ti + 1) * P],
                            ident[0:H, 0:H],
                        )
                        nc.vector.tensor_copy(v_sb[:, ti, 0:H], psv[:])

                prev = None  # (tcn, xt_sb) one chunk behind
                for tcn in range(NTC):
                    xts = []
                    for tt in range(4):
                        xt_in = xpool.tile([P, E], F32R, tag="x")
                        t0 = (tcn * 4 + tt) * P
                        nc.sync.dma_start(xt_in[:], x[t0 : t0 + P, :])
                        xts.append(xt_in)
                    xt_sb = xtpool.tile([P, NE, TC], F32R, tag="xt")
                    for ec in range(NE):
                        pxt = ps_xt.tile([P, 4 * P], F32, tag="pxt")
                        for tt in range(4):
                            nc.tensor.transpose(
                                pxt[:, tt * P : (tt + 1) * P].bitcast(F32R),
                                xts[tt][:, ec * P : (ec + 1) * P],
                                ident,
                            )
                        # split the PSUM->SBUF casts between DVE and ScalarE
                        if ec % 2 == 0:
                            nc.vector.tensor_copy(xt_sb[:, ec, :], pxt[:])
                        else:
                            nc.scalar.copy(xt_sb[:, ec, :], pxt[:])
                    if prev is not None:
                        emit_proj(*prev)
                    prev = (tcn, xt_sb)
                emit_proj(*prev)
                # one DMA for the K^T re-base: all phase-B S matmuls then
                # depend on a single queue semaphore
                nc.gpsimd.dma_start(kt_sb[:], qk_sb[H : 2 * H, :])

            # ---------------- phase B: scores, softmax, PV ----------------
            with (
                tc.tile_pool(name="es", bufs=3) as espool,
                tc.tile_pool(name="ps_s", bufs=4, space="PSUM") as ps_s,
                tc.tile_pool(name="ps_o", bufs=1, space="PSUM") as ps_o,
            ):
                # PE observes the last phase-A DVE tick (psv copy of tile 15),
                # covering the PSUM WAR deps of the re-used banks below
                bridge = ps_s.tile([P, TC], F32, tag="s")
                nc.tensor.matmul(
                    bridge[0:H, 0:H],
                    v_sb[:, NT - 1, 0:H],
                    v_sb[:, NT - 1, 0:H],
                    start=True,
                    stop=True,
                )
                o_ps = [
                    ps_o.tile([H + 1, TC], F32, tag=f"o{c}", name=f"o_ps{c}")
                    for c in range(NTC)
                ]
                scale = 1.0 / np.sqrt(np.float32(H))
                es_tiles = {}

                def emit_scores(j):
                    k0 = j * P
                    es = espool.tile([P, T], F32R, tag="es", name=f"es{j}")
                    es_tiles[j] = es
                    for c in range(j // 4, NTC):
                        q0 = max(c * TC, k0)
                        w = (c + 1) * TC - q0
                        ps = ps_s.tile([P, TC], F32, tag="s", name=f"s{j}_{c}")
                        nc.tensor.matmul(
                            ps[:, :w],
                            kt_sb[:, k0 : k0 + P],
                            qk_sb[0:H, q0 : q0 + w],
                            start=True,
                            stop=True,
                        )
                        if c == j // 4:
                            # causal mask inside the diagonal 128x128 block:
                            # add -1e30 where y < p, in PSUM, before the exp
                            nc.vector.tensor_tensor(
                                ps[:, 0:P],
                                ps[:, 0:P],
                                maskneg,
                                mybir.AluOpType.add,
                            )
                        nc.scalar.activation(
                            es[:, q0 : q0 + w], ps[:, :w], AF.Exp, scale=float(scale)
                        )

                def emit_pv(j):
                    k0 = j * P
                    es = es_tiles.pop(j)
                    for c in range(j // 4, NTC):
                        q0 = max(c * TC, k0)
                        w = (c + 1) * TC - q0
                        nc.tensor.matmul(
                            o_ps[c][:, q0 - c * TC : q0 - c * TC + w],
                            v_sb[:, j, :],
                            es[:, q0 : q0 + w],
                            start=(j == 0),
                            stop=(j == 4 * c + 3),
                        )
                        if j == 4 * c + 3:
                            nc.vector.tensor_copy(
                                o_sb[:, c * TC : (c + 1) * TC], o_ps[c][:]
                            )
                            nc.sync.dma_start(
                                out[:, c * TC : (c + 1) * TC],
                                o_sb[:, c * TC : (c + 1) * TC],
                            )

                # software pipeline: PV of chunk j-1 runs while exp of chunk j
                # is still in flight, so the PE never waits on ScalarE
                emit_scores(0)
                for j in range(1, NT):
                    emit_scores(j)
                    emit_pv(j - 1)
                emit_pv(NT - 1)
    nc.compile()
    return nc


_NC_CACHE = None


def _get_nc():
    global _NC_CACHE
    if _NC_CACHE is None:
        _NC_CACHE = build_kernel()
    return _NC_CACHE


def kernel(batch_x, Wk, bk, Wq, bq, Wv, bv):
    batch_x = np.ascontiguousarray(np.asarray(batch_x, dtype=np.float32))
    cbk = pack_const_block(
        np.asarray(Wq, dtype=np.float32),
        np.asarray(Wk, dtype=np.float32),
        np.asarray(Wv, dtype=np.float32),
        np.asarray(bq, dtype=np.float32),
        np.asarray(bk, dtype=np.float32),
        np.asarray(bv, dtype=np.float32),
    )
    nc = _get_nc()
    in_maps = [{"x": batch_x[i], "cb": cbk} for i in range(NCORES)]
    res = run_bass_kernel_spmd(nc, in_maps, list(range(NCORES)))
    outs = []
    for i in range(NCORES):
        o = res.results[i]["out"]  # [65, 2048]
        outs.append((o[:H] / o[H : H + 1]).T)  # normalize + transpose
    return np.stack(outs).astype(np.float32)


if __name__ == "__main__":
    rng = np.random.default_rng(0)
    inputs = {
        "batch_x": rng.standard_normal((NCORES, T, E), dtype=np.float32),
        "Wk": rng.standard_normal((E, H), dtype=np.float32) * 0.03,
        "bk": rng.standard_normal((H,), dtype=np.float32) * 0.03,
        "Wq": rng.standard_normal((E, H), dtype=np.float32) * 0.03,
        "bq": rng.standard_normal((H,), dtype=np.float32) * 0.03,
        "Wv": rng.standard_normal((E, H), dtype=np.float32) * 0.03,
        "bv": rng.standard_normal((H,), dtype=np.float32) * 0.03,
    }
    out = kernel(**inputs)
    print(out.shape, out.dtype)


# revision 24
# speedup vs baseline: 1.3859x; 1.2685x over previous
"""Trainium2 Bass kernel: single-head causal attention (B=8, T=2048, E=1024, H=64).

Sharding: data-parallel over the batch dim — one batch element per NeuronCore,
8 cores, no collectives.

Per-core pipeline (matmuls in float32r — full PE rate at N>=256):
  phase A: DMA X [T,E] by 128-row tiles; PE-transpose (f32r, 1.5 cyc/row) into
           XT [E,T] chunks (PSUM->SBUF copies split DVE/ScalarE);
           projections QK^T = [Wq|Wk]^T @ X^T (one M=128 matmul) and
           V^T = Wv^T @ X^T run one t-chunk BEHIND the transposes so the PE
           never waits on the copies. Biases fused into the PSUM->SBUF copies.
           V^T is PE-transposed back to natural [t, H] layout with a ones
           column appended (V').
  phase B: software-pipelined per 128-wide k-chunk j: S^T_j = K_j Q^T for
           q >= 128j (causal skip) while PV of chunk j-1 runs, so the PE never
           waits on the exp. Causal -1e30 mask added to the diagonal 128x128
           block in PSUM by DVE pre-exp; exp on ScalarE straight from PSUM
           with the 1/sqrt(H) scale fused; P~V accumulates into O' PSUM
           [65, q] whose row 64 (ones column of V') is the denominator Z.
  Output per core: [65, 2048] = [unnormalized O^T; Z]. Host divides by Z and
  transposes during the unshard (part of gather).

Hardware quirks catered to:
  - f32r matmul operands must be produced rounded-to-f32r (verifier rule);
    DMA f32r->f32r is accepted, so x/cb ship as f32r from the host.
  - f32r ISA restrictions: moving-operand and dst innermost counts even,
    dst 8B-aligned at partition 0.
  - PE matmuls carry only ONE semaphore wait (walrus limit) — bacc's
    generate_event_semaphores splits the rest, but the kernel is structured
    to keep cross-engine fan-in low anyway.
  - A warmup matmul burst during the DMA prologue brings the PE HAM clock
    gate to 2.4 GHz before the real work starts.
"""

import numpy as np

import concourse.bass as bass
import concourse.bacc as bacc
import concourse.mybir as mybir
from concourse.tile import TileContext
from concourse.bass_utils import run_bass_kernel_spmd

T = 2048
E = 1024
H = 64
P = 128
TC = 512  # t/q chunk width (one PSUM bank of f32)
NT = T // P  # 16 t-tiles
NE = E // P  # 8 e-chunks
NTC = T // TC  # 4 t-chunks
NCORES = 8

F32 = mybir.dt.float32
F32R = mybir.dt.float32r
AF = mybir.ActivationFunctionType

# constant block column layout (per partition)
CB_IDENT = 0  # [128] identity
CB_WQK = CB_IDENT + P  # [NE * 2H] = 1024, [e_chunk, m] with m: 0:64=Wq, 64:128=Wk
CB_WV = CB_WQK + NE * 2 * H  # [NE * H] = 512
CB_MASK = CB_WV + NE * H  # [128] causal mask: 0 keep (y>=p), -1e30 drop
CB_BQK = CB_MASK + P  # [1] bq on partitions 0:64, bk on 64:128
CB_BV = CB_BQK + 1  # [1] bv on partitions 0:64
CB_COLS = CB_BV + 1


def pack_const_block(Wq, Wk, Wv, bq, bk, bv):
    cb = np.zeros((P, CB_COLS), dtype=np.float32)
    cb[:, CB_IDENT : CB_IDENT + P] = np.eye(P, dtype=np.float32)
    wqk = np.zeros((P, NE, 2 * H), dtype=np.float32)
    wqk[:, :, 0:H] = Wq.reshape(NE, P, H).transpose(1, 0, 2)
    wqk[:, :, H : 2 * H] = Wk.reshape(NE, P, H).transpose(1, 0, 2)
    cb[:, CB_WQK : CB_WQK + NE * 2 * H] = wqk.reshape(P, NE * 2 * H)
    cb[:, CB_WV : CB_WV + NE * H] = (
        Wv.reshape(NE, P, H).transpose(1, 0, 2).reshape(P, NE * H)
    )
    p_idx = np.arange(P)[:, None]
    y_idx = np.arange(P)[None, :]
    cb[:, CB_MASK : CB_MASK + P] = np.where(y_idx >= p_idx, 0.0, -1e30).astype(
        np.float32
    )
    cb[0:H, CB_BQK] = bq
    cb[H : 2 * H, CB_BQK] = bk
    cb[0:H, CB_BV] = bv
    return cb


def build_kernel():
    nc = bacc.Bacc("TRN2", target_bir_lowering=False, debug=False)
    x = nc.dram_tensor("x", [T, E], F32R, kind="ExternalInput")
    cb = nc.dram_tensor("cb", [P, CB_COLS], F32R, kind="ExternalInput")
    out = nc.dram_tensor("out", [H + 1, T], F32, kind="ExternalOutput")

    with TileContext(nc) as tc:
        with tc.tile_pool(name="const", bufs=1) as const:
            cb_sb = const.tile([P, CB_COLS], F32R)
            nc.sync.dma_start(cb_sb[:], cb[:])
            ident = cb_sb[:, CB_IDENT : CB_IDENT + P]  # f32r identity
            wqk_sb = cb_sb[:, CB_WQK : CB_WQK + NE * 2 * H].rearrange(
                "p (c m) -> p c m", m=2 * H
            )
            wv_sb = cb_sb[:, CB_WV : CB_WV + NE * H].rearrange(
                "p (c m) -> p c m", m=H
            )
            maskneg = cb_sb[:, CB_MASK : CB_MASK + P].bitcast(F32)
            bqk_t = cb_sb[:, CB_BQK : CB_BQK + 1].bitcast(F32)
            bv_t = cb_sb[0:H, CB_BV : CB_BV + 1].bitcast(F32)

            # persistent activations
            qk_sb = const.tile([P, T], F32R)  # rows 0:64 = Q^T, 64:128 = K^T
            kt_sb = const.tile([P, T], F32R)  # K^T re-based, rows 64:128 zero-padded
            vt_sb = const.tile([H, T], F32R)  # V^T staging
            v_sb = const.tile([P, NT, H + 1], F32R)  # V' = [V, 1] natural layout
            o_sb = const.tile([H + 1, T], F32)
            # zero-pad kt_sb rows 64:128 so S^T matmuls contract over all 128
            # partitions (keeps the PE HAM activity monitor at full clock)
            nc.vector.tensor_scalar(
                kt_sb[H : 2 * H, 0:1024],
                cb_sb[0:H, 0:1024],
                0.0,
                0.0,
                mybir.AluOpType.mult,
                mybir.AluOpType.add,
            )
            nc.vector.tensor_scalar(
                kt_sb[H : 2 * H, 1024:2048],
                cb_sb[0:H, 0:1024],
                0.0,
                0.0,
                mybir.AluOpType.mult,
                mybir.AluOpType.add,
            )
            # ones column of V' (memset can't write f32r; 0*x+1 rounds via DVE)
            nc.vector.tensor_scalar(
                v_sb[:, :, H],
                cb_sb[:, 0:NT],
                0.0,
                1.0,
                mybir.AluOpType.mult,
                mybir.AluOpType.add,
            )

            # ---------------- phase A: transpose X + projections ----------------
            with (
                tc.tile_pool(name="xin", bufs=9) as xpool,
                tc.tile_pool(name="xt", bufs=2) as xtpool,
                tc.tile_pool(name="ps_xt", bufs=3, space="PSUM") as ps_xt,
                tc.tile_pool(name="ps_prj", bufs=2, space="PSUM") as ps_prj,
                tc.tile_pool(name="ps_v", bufs=1, space="PSUM") as ps_v,
            ):
                # HAM warmup: ~4.5us of back-to-back matmuls while the first
                # x tiles stream in, so real work starts at 2.4 GHz
                warm = ps_xt.tile([P, 4 * P], F32, tag="pxt")
                for _ in range(16):
                    nc.tensor.matmul(
                        warm[:, 0:P],
                        cb_sb[:, 0:P],
                        cb_sb[:, 0:P],
                        start=True,
                        stop=True,
                    )

                def emit_proj(tcn, xt_sb):
                    pqk = ps_prj.tile([P, TC], F32, tag="pqk", name=f"pqk{tcn}")
                    pv = ps_prj.tile([H, TC], F32, tag="pv", name=f"pv{tcn}")
                    for ec in range(NE):
                        nc.tensor.matmul(
                            pqk[:],
                            wqk_sb[:, ec, :],
                            xt_sb[:, ec, :],
                            start=(ec == 0),
                            stop=(ec == NE - 1),
                        )
                    for ec in range(NE):
                        nc.tensor.matmul(
                            pv[:],
                            wv_sb[:, ec, :],
                            xt_sb[:, ec, :],
                            start=(ec == 0),
                            stop=(ec == NE - 1),
                        )
                    c0 = tcn * TC
                    nc.vector.tensor_scalar_add(qk_sb[:, c0 : c0 + TC], pqk[:], bqk_t)
                    nc.gpsimd.dma_start(
                        kt_sb[0:H, c0 : c0 + TC], qk_sb[H : 2 * H, c0 : c0 + TC]
                    )
                    nc.vector.tensor_scalar_add(vt_sb[:, c0 : c0 + TC], pv[:], bv_t)
                    for tt in range(4):
                        ti = tcn * 4 + tt
                        psv = ps_v.tile([P, H], F32, tag="psv", name=f"psv{ti}")
                        nc.tensor.transpose(
                            psv[:].bitcast(F32R),
                            vt_sb[:, ti * P : (ti + 1) * P],
                            ident[0:H, 0:H],
                        )
                        nc.vector.tensor_copy(v_sb[:, ti, 0:H], psv[:])

                prev = None  # (tcn, xt_sb) one chunk behind
                for tcn in range(NTC):
                    xts = []
                    for tt in range(4):
                        xt_in = xpool.tile([P, E], F32R, tag="x")
                        t0 = (tcn * 4 + tt) * P
                        nc.sync.dma_start(xt_in[:], x[t0 : t0 + P, :])
                        xts.append(xt_in)
                    xt_sb = xtpool.tile([P, NE, TC], F32R, tag="xt")
                    for ec in range(NE):
                        pxt = ps_xt.tile([P, 4 * P], F32, tag="pxt")
                        for tt in range(4):
                            nc.tensor.transpose(
                                pxt[:, tt * P : (tt + 1) * P].bitcast(F32R),
                                xts[tt][:, ec * P : (ec + 1) * P],
                                ident,
                            )
                        # split the PSUM->SBUF casts between DVE and ScalarE
                        if ec % 2 == 0:
                            nc.vector.tensor_copy(xt_sb[:, ec, :], pxt[:])
                        else:
                            nc.scalar.copy(xt_sb[:, ec, :], pxt[:])
                    if prev is not None:
                        emit_proj(*prev)
                    prev = (tcn, xt_sb)
                emit_proj(*prev)

            # ---------------- phase B: scores, softmax, PV ----------------
            with (
                tc.tile_pool(name="es", bufs=3) as espool,
                tc.tile_pool(name="ps_s", bufs=4, space="PSUM") as ps_s,
                tc.tile_pool(name="ps_o", bufs=1, space="PSUM") as ps_o,
            ):
                # PE observes the last phase-A DVE tick (psv copy of tile 15),
                # covering the PSUM WAR deps of the re-used banks below
                bridge = ps_s.tile([P, TC], F32, tag="s")
                nc.tensor.matmul(
                    bridge[0:H, 0:H],
                    v_sb[:, NT - 1, 0:H],
                    v_sb[:, NT - 1, 0:H],
                    start=True,
                    stop=True,
                )
                o_ps = [
                    ps_o.tile([H + 1, TC], F32, tag=f"o{c}", name=f"o_ps{c}")
                    for c in range(NTC)
                ]
                scale = 1.0 / np.sqrt(np.float32(H))
                es_tiles = {}

                def emit_scores(j):
                    k0 = j * P
                    es = espool.tile([P, T], F32R, tag="es", name=f"es{j}")
                    es_tiles[j] = es
                    for c in range(j // 4, NTC):
                        q0 = max(c * TC, k0)
                        w = (c + 1) * TC - q0
                        ps = ps_s.tile([P, TC], F32, tag="s", name=f"s{j}_{c}")
                        nc.tensor.matmul(
                            ps[:, :w],
                            kt_sb[:, k0 : k0 + P],
                            qk_sb[:, q0 : q0 + w],
                            start=True,
                            stop=True,
                        )
                        if c == j // 4:
                            # causal mask inside the diagonal 128x128 block:
                            # add -1e30 where y < p, in PSUM, before the exp
                            nc.vector.tensor_tensor(
                                ps[:, 0:P],
                                ps[:, 0:P],
                                maskneg,
                                mybir.AluOpType.add,
                            )
                        nc.scalar.activation(
                            es[:, q0 : q0 + w], ps[:, :w], AF.Exp, scale=float(scale)
                        )

                def emit_pv(j):
                    k0 = j * P
                    es = es_tiles.pop(j)
                    for c in range(j // 4, NTC):
                        q0 = max(c * TC, k0)
                        w = (c + 1) * TC - q0
                        nc.tensor.matmul(
                            o_ps[c][:, q0 - c * TC : q0 - c * TC + w],
                            v_sb[:, j, :],
                            es[:, q0 : q0 + w],
                            start=(j == 0),
                            stop=(j == 4 * c + 3),
                        )
                        if j == 4 * c + 3:
                            nc.vector.tensor_copy(
                                o_sb[:, c * TC : (c + 1) * TC], o_ps[c][:]
                            )
                            nc.sync.dma_start(
                                out[:, c * TC : (c + 1) * TC],
                                o_sb[:, c * TC : (c + 1) * TC],
                            )

                # software pipeline: PV of chunk j-1 runs while exp of chunk j
                # is still in flight, so the PE never waits on ScalarE
                emit_scores(0)
                for j in range(1, NT):
                    emit_scores(j)
                    emit_pv(j - 1)
                emit_pv(NT - 1)
    nc.compile()
    return nc


_NC_CACHE = None


def _get_nc():
    global _NC_CACHE
    if _NC_CACHE is None:
        _NC_CACHE = build_kernel()
    return _NC_CACHE


def kernel(batch_x, Wk, bk, Wq, bq, Wv, bv):
    batch_x = np.ascontiguousarray(np.asarray(batch_x, dtype=np.float32))
    cbk = pack_const_block(
        np.asarray(Wq, dtype=np.float32),
        np.asarray(Wk, dtype=np.float32),
        np.asarray(Wv, dtype=np.float32),
        np.asarray(bq, dtype=np.float32),
        np.asarray(bk, dtype=np.float32),
        np.asarray(bv, dtype=np.float32),
    )
    nc = _get_nc()
    in_maps = [{"x": batch_x[i], "cb": cbk} for i in range(NCORES)]
    res = run_bass_kernel_spmd(nc, in_maps, list(range(NCORES)))
    outs = []
    for i in range(NCORES):
        o = res.results[i]["out"]  # [65, 2048]
        outs.append((o[:H] / o[H : H + 1]).T)  # normalize + transpose
    return np.stack(outs).astype(np.float32)


if __name__ == "__main__":
    rng = np.random.default_rng(0)
    inputs = {
        "batch_x": rng.standard_normal((NCORES, T, E), dtype=np.float32),
        "Wk": rng.standard_normal((E, H), dtype=np.float32) * 0.03,
        "bk": rng.standard_normal((H,), dtype=np.float32) * 0.03,
        "Wq": rng.standard_normal((E, H), dtype=np.float32) * 0.03,
        "bq": rng.standard_normal((H,), dtype=np.float32) * 0.03,
        "Wv": rng.standard_normal((E, H), dtype=np.float32) * 0.03,
        "bv": rng.standard_normal((H,), dtype=np.float32) * 0.03,
    }
    out = kernel(**inputs)
    print(out.shape, out.dtype)


# revision 25
# speedup vs baseline: 1.4337x; 1.0345x over previous
"""Trainium2 Bass kernel: single-head causal attention (B=8, T=2048, E=1024, H=64).

Sharding: data-parallel over the batch dim — one batch element per NeuronCore,
8 cores, no collectives.

Per-core pipeline (matmuls in float32r — full PE rate at N>=256):
  phase A: DMA X [T,E] by 128-row tiles; PE-transpose (f32r, 1.5 cyc/row) into
           XT [E,T] chunks (PSUM->SBUF copies split DVE/ScalarE);
           projections QK^T = [Wq|Wk]^T @ X^T (one M=128 matmul) and
           V^T = Wv^T @ X^T run one t-chunk BEHIND the transposes so the PE
           never waits on the copies. Biases fused into the PSUM->SBUF copies.
           V^T is PE-transposed back to natural [t, H] layout with a ones
           column appended (V').
  phase B: software-pipelined per 128-wide k-chunk j: S^T_j = K_j Q^T for
           q >= 128j (causal skip) while PV of chunk j-1 runs, so the PE never
           waits on the exp. Causal -1e30 mask added to the diagonal 128x128
           block in PSUM by DVE pre-exp; exp on ScalarE straight from PSUM
           with the 1/sqrt(H) scale fused; P~V accumulates into O' PSUM
           [65, q] whose row 64 (ones column of V') is the denominator Z.
  Output per core: [65, 2048] = [unnormalized O^T; Z]. Host divides by Z and
  transposes during the unshard (part of gather).

Hardware quirks catered to:
  - f32r matmul operands must be produced rounded-to-f32r (verifier rule);
    DMA f32r->f32r is accepted, so x/cb ship as f32r from the host.
  - f32r ISA restrictions: moving-operand and dst innermost counts even,
    dst 8B-aligned at partition 0.
  - PE matmuls carry only ONE semaphore wait (walrus limit) — bacc's
    generate_event_semaphores splits the rest, but the kernel is structured
    to keep cross-engine fan-in low anyway.
  - A warmup matmul burst during the DMA prologue brings the PE HAM clock
    gate to 2.4 GHz before the real work starts.
"""

import numpy as np

import concourse.bass as bass
import concourse.bacc as bacc
import concourse.mybir as mybir
from concourse.tile import TileContext
from concourse.bass_utils import run_bass_kernel_spmd

T = 2048
E = 1024
H = 64
P = 128
TC = 512  # t/q chunk width (one PSUM bank of f32)
NT = T // P  # 16 t-tiles
NE = E // P  # 8 e-chunks
NTC = T // TC  # 4 t-chunks
NCORES = 8

F32 = mybir.dt.float32
F32R = mybir.dt.float32r
AF = mybir.ActivationFunctionType

# constant block column layout (per partition)
CB_IDENT = 0  # [128] identity
CB_WQK = CB_IDENT + P  # [NE * 2H] = 1024, [e_chunk, m] with m: 0:64=Wq, 64:128=Wk
CB_WV = CB_WQK + NE * 2 * H  # [NE * H] = 512
CB_MASK = CB_WV + NE * H  # [128] causal mask: 0 keep (y>=p), -1e30 drop
CB_BQK = CB_MASK + P  # [1] bq on partitions 0:64, bk on 64:128
CB_BV = CB_BQK + 1  # [1] bv on partitions 0:64
CB_COLS = CB_BV + 1


def pack_const_block(Wq, Wk, Wv, bq, bk, bv):
    cb = np.zeros((P, CB_COLS), dtype=np.float32)
    cb[:, CB_IDENT : CB_IDENT + P] = np.eye(P, dtype=np.float32)
    wqk = np.zeros((P, NE, 2 * H), dtype=np.float32)
    wqk[:, :, 0:H] = Wq.reshape(NE, P, H).transpose(1, 0, 2)
    wqk[:, :, H : 2 * H] = Wk.reshape(NE, P, H).transpose(1, 0, 2)
    cb[:, CB_WQK : CB_WQK + NE * 2 * H] = wqk.reshape(P, NE * 2 * H)
    cb[:, CB_WV : CB_WV + NE * H] = (
        Wv.reshape(NE, P, H).transpose(1, 0, 2).reshape(P, NE * H)
    )
    p_idx = np.arange(P)[:, None]
    y_idx = np.arange(P)[None, :]
    cb[:, CB_MASK : CB_MASK + P] = np.where(y_idx >= p_idx, 0.0, -1e30).astype(
        np.float32
    )
    cb[0:H, CB_BQK] = bq
    cb[H : 2 * H, CB_BQK] = bk
    cb[0:H, CB_BV] = bv
    return cb


def build_kernel():
    nc = bacc.Bacc("TRN2", target_bir_lowering=False, debug=False)
    x = nc.dram_tensor("x", [T, E], F32R, kind="ExternalInput")
    cb = nc.dram_tensor("cb", [P, CB_COLS], F32R, kind="ExternalInput")
    out = nc.dram_tensor("out", [H + 1, T], F32, kind="ExternalOutput")

    with TileContext(nc) as tc:
        with tc.tile_pool(name="const", bufs=1) as const:
            cb_sb = const.tile([P, CB_COLS], F32R)
            nc.sync.dma_start(cb_sb[:], cb[:])
            ident = cb_sb[:, CB_IDENT : CB_IDENT + P]  # f32r identity
            wqk_sb = cb_sb[:, CB_WQK : CB_WQK + NE * 2 * H].rearrange(
                "p (c m) -> p c m", m=2 * H
            )
            wv_sb = cb_sb[:, CB_WV : CB_WV + NE * H].rearrange(
                "p (c m) -> p c m", m=H
            )
            maskneg = cb_sb[:, CB_MASK : CB_MASK + P].bitcast(F32)
            bqk_t = cb_sb[:, CB_BQK : CB_BQK + 1].bitcast(F32)
            bv_t = cb_sb[0:H, CB_BV : CB_BV + 1].bitcast(F32)

            # persistent activations
            qk_sb = const.tile([P, T], F32R)  # rows 0:64 = Q^T, 64:128 = K^T
            kt_sb = const.tile([P, T], F32R)  # K^T re-based, rows 64:128 zero-padded
            vt_sb = const.tile([H, T], F32R)  # V^T staging
            v_sb = const.tile([P, NT, H + 1], F32R)  # V' = [V, 1] natural layout
            o_sb = const.tile([H + 1, T], F32)
            # zero-pad kt_sb rows 64:128 so S^T matmuls contract over all 128
            # partitions (keeps the PE HAM activity monitor at full clock)
            nc.vector.tensor_scalar(
                kt_sb[H : 2 * H, 0:1024],
                cb_sb[0:H, 0:1024],
                0.0,
                0.0,
                mybir.AluOpType.mult,
                mybir.AluOpType.add,
            )
            nc.vector.tensor_scalar(
                kt_sb[H : 2 * H, 1024:2048],
                cb_sb[0:H, 0:1024],
                0.0,
                0.0,
                mybir.AluOpType.mult,
                mybir.AluOpType.add,
            )
            # ones column of V' (memset can't write f32r; 0*x+1 rounds via DVE)
            nc.vector.tensor_scalar(
                v_sb[:, :, H],
                cb_sb[:, 0:NT],
                0.0,
                1.0,
                mybir.AluOpType.mult,
                mybir.AluOpType.add,
            )

            # ---------------- phase A: transpose X + projections ----------------
            with (
                tc.tile_pool(name="xin", bufs=9) as xpool,
                tc.tile_pool(name="xt", bufs=2) as xtpool,
                tc.tile_pool(name="ps_xt", bufs=3, space="PSUM") as ps_xt,
                tc.tile_pool(name="ps_prj", bufs=2, space="PSUM") as ps_prj,
                tc.tile_pool(name="ps_v", bufs=1, space="PSUM") as ps_v,
            ):
                # HAM warmup: ~4.5us of back-to-back matmuls while the first
                # x tiles stream in, so real work starts at 2.4 GHz
                warm = ps_xt.tile([P, 4 * P], F32, tag="pxt")
                for _ in range(16):
                    nc.tensor.matmul(
                        warm[:, 0:P],
                        cb_sb[:, 0:P],
                        cb_sb[:, 0:P],
                        start=True,
                        stop=True,
                    )

                def emit_proj(tcn, xt_sb):
                    pqk = ps_prj.tile([P, TC], F32, tag="pqk", name=f"pqk{tcn}")
                    pv = ps_prj.tile([H, TC], F32, tag="pv", name=f"pv{tcn}")
                    for ec in range(NE):
                        nc.tensor.matmul(
                            pqk[:],
                            wqk_sb[:, ec, :],
                            xt_sb[:, ec, :],
                            start=(ec == 0),
                            stop=(ec == NE - 1),
                        )
                    for ec in range(NE):
                        nc.tensor.matmul(
                            pv[:],
                            wv_sb[:, ec, :],
                            xt_sb[:, ec, :],
                            start=(ec == 0),
                            stop=(ec == NE - 1),
                        )
                    c0 = tcn * TC
                    nc.vector.tensor_scalar_add(qk_sb[:, c0 : c0 + TC], pqk[:], bqk_t)
                    nc.gpsimd.dma_start(
                        kt_sb[0:H, c0 : c0 + TC], qk_sb[H : 2 * H, c0 : c0 + TC]
                    )
                    nc.vector.tensor_scalar_add(vt_sb[:, c0 : c0 + TC], pv[:], bv_t)
                    for tt in range(4):
                        ti = tcn * 4 + tt
                        psv = ps_v.tile([P, H], F32, tag="psv", name=f"psv{ti}")
                        nc.tensor.transpose(
                            psv[:].bitcast(F32R),
                            vt_sb[:, ti * P : (ti + 1) * P],
                            ident[0:H, 0:H],
                        )
                        nc.vector.tensor_copy(v_sb[:, ti, 0:H], psv[:])

                prev = None  # (tcn, xt_sb) one chunk behind
                for tcn in range(NTC):
                    xts = []
                    for tt in range(4):
                        xt_in = xpool.tile([P, E], F32R, tag="x")
                        t0 = (tcn * 4 + tt) * P
                        npieces = 4 if (tcn == 0 and tt == 0) else 2
                        step = E // npieces
                        for pc in range(npieces):
                            nc.sync.dma_start(
                                xt_in[:, pc * step : (pc + 1) * step],
                                x[t0 : t0 + P, pc * step : (pc + 1) * step],
                            )
                        xts.append(xt_in)
                    xt_sb = xtpool.tile([P, NE, TC], F32R, tag="xt")
                    for ec in range(NE):
                        pxt = ps_xt.tile([P, 4 * P], F32, tag="pxt")
                        for tt in range(4):
                            nc.tensor.transpose(
                                pxt[:, tt * P : (tt + 1) * P].bitcast(F32R),
                                xts[tt][:, ec * P : (ec + 1) * P],
                                ident,
                            )
                        nc.vector.tensor_copy(xt_sb[:, ec, :], pxt[:])
                    if prev is not None:
                        emit_proj(*prev)
                    prev = (tcn, xt_sb)
                emit_proj(*prev)

            # ---------------- phase B: scores, softmax, PV ----------------
            with (
                tc.tile_pool(name="es", bufs=3) as espool,
                tc.tile_pool(name="ps_s", bufs=4, space="PSUM") as ps_s,
                tc.tile_pool(name="ps_o", bufs=1, space="PSUM") as ps_o,
            ):
                # PE observes the last phase-A DVE tick (psv copy of tile 15),
                # covering the PSUM WAR deps of the re-used banks below
                bridge = ps_s.tile([P, TC], F32, tag="s")
                nc.tensor.matmul(
                    bridge[0:H, 0:H],
                    v_sb[:, NT - 1, 0:H],
                    v_sb[:, NT - 1, 0:H],
                    start=True,
                    stop=True,
                )
                o_ps = [
                    ps_o.tile([H + 1, TC], F32, tag=f"o{c}", name=f"o_ps{c}")
                    for c in range(NTC)
                ]
                scale = 1.0 / np.sqrt(np.float32(H))
                es_tiles = {}

                def emit_scores(j):
                    k0 = j * P
                    es = espool.tile([P, T], F32R, tag="es", name=f"es{j}")
                    es_tiles[j] = es
                    for c in range(j // 4, NTC):
                        q0 = max(c * TC, k0)
                        w = (c + 1) * TC - q0
                        ps = ps_s.tile([P, TC], F32, tag="s", name=f"s{j}_{c}")
                        nc.tensor.matmul(
                            ps[:, :w],
                            kt_sb[:, k0 : k0 + P],
                            qk_sb[:, q0 : q0 + w],
                            start=True,
                            stop=True,
                        )
                        if c == j // 4:
                            # causal mask inside the diagonal 128x128 block:
                            # add -1e30 where y < p, in PSUM, before the exp
                            nc.vector.tensor_tensor(
                                ps[:, 0:P],
                                ps[:, 0:P],
                                maskneg,
                                mybir.AluOpType.add,
                            )
                        nc.scalar.activation(
                            es[:, q0 : q0 + w], ps[:, :w], AF.Exp, scale=float(scale)
                        )

                def emit_pv(j):
                    k0 = j * P
                    es = es_tiles.pop(j)
                    for c in range(j // 4, NTC):
                        q0 = max(c * TC, k0)
                        w = (c + 1) * TC - q0
                        nc.tensor.matmul(
                            o_ps[c][:, q0 - c * TC : q0 - c * TC + w],
                            v_sb[:, j, :],
                            es[:, q0 : q0 + w],
                            start=(j == 0),
                            stop=(j == 4 * c + 3),
                        )
                        if j == 4 * c + 3:
                            nc.vector.tensor_copy(
                                o_sb[:, c * TC : (c + 1) * TC], o_ps[c][:]
                            )
                            nc.sync.dma_start(
                                out[:, c * TC : (c + 1) * TC],
                                o_sb[:, c * TC : (c + 1) * TC],
                            )

                # software pipeline: PV of chunk j-1 runs while exp of chunk j
                # is still in flight, so the PE never waits on ScalarE
                emit_scores(0)
                for j in range(1, NT):
                    emit_scores(j)
                    emit_pv(j - 1)
                emit_pv(NT - 1)
    nc.compile()
    return nc


_NC_CACHE = None


def _get_nc():
    global _NC_CACHE
    if _NC_CACHE is None:
        _NC_CACHE = build_kernel()
    return _NC_CACHE


def kernel(batch_x, Wk, bk, Wq, bq, Wv, bv):
    batch_x = np.ascontiguousarray(np.asarray(batch_x, dtype=np.float32))
    cbk = pack_const_block(
        np.asarray(Wq, dtype=np.float32),
        np.asarray(Wk, dtype=np.float32),
        np.asarray(Wv, dtype=np.float32),
        np.asarray(bq, dtype=np.float32),
        np.asarray(bk, dtype=np.float32),
        np.asarray(bv, dtype=np.float32),
    )
    nc = _get_nc()
    in_maps = [{"x": batch_x[i], "cb": cbk} for i in range(NCORES)]
    res = run_bass_kernel_spmd(nc, in_maps, list(range(NCORES)))
    outs = []
    for i in range(NCORES):
        o = res.results[i]["out"]  # [65, 2048]
        outs.append((o[:H] / o[H : H + 1]).T)  # normalize + transpose
    return np.stack(outs).astype(np.float32)


if __name__ == "__main__":
    rng = np.random.default_rng(0)
    inputs = {
        "batch_x": rng.standard_normal((NCORES, T, E), dtype=np.float32),
        "Wk": rng.standard_normal((E, H), dtype=np.float32) * 0.03,
        "bk": rng.standard_normal((H,), dtype=np.float32) * 0.03,
        "Wq": rng.standard_normal((E, H), dtype=np.float32) * 0.03,
        "bq": rng.standard_normal((H,), dtype=np.float32) * 0.03,
        "Wv": rng.standard_normal((E, H), dtype=np.float32) * 0.03,
        "bv": rng.standard_normal((H,), dtype=np.float32) * 0.03,
    }
    out = kernel(**inputs)
    print(out.shape, out.dtype)
